# revision 12
# baseline (speedup 1.0000x reference)
"""Trainium2 Bass kernel for a 2-layer GCN (nn_ClusterGNN):
    h1 = relu(gcn_conv(x, W1, b1)); out = log_softmax(gcn_conv(h1, W2, b2))

Strategy (8 NeuronCores, dest-node sharded):
  - The GCN symmetric norm factorizes: msg(r->c) = dinv[r]*dinv[c]*h[r].
    dinv[src] is folded into x on the host, the dinv[dst] factor is
    deferred into downstream per-partition scales, so edge aggregation
    is a pure unweighted segment-sum.
  - Segment-sum runs on the tensor engine: per 128-edge block, a one-hot
    selector S[e, d] = (col_local[e] == d) is built with one
    tensor_scalar(is_equal) in fp16 (split across DVE and Pool engines),
    then layer 1 accumulates G^T[f, 128] += msg[128, F].T @ S[128, 128]
    and layer 2 accumulates G[128, F] += S[128, 128].T @ msg[128, F]
    (selector as stationary weights; Ldweights is free, so the matmul
    cost is the output free dim: 32 instead of 128).
  - Edge messages come from raw InstDMAGatherAnt (fp16 rows, 128B for
    layer 1 / 64B for layer 2, 256B row stride) out of a per-core
    replicated table; int16 indices are window-local (32768-row windows).
    Gathers are issued one chunk ahead of compute to keep the DMA
    engines busy.
  - Layer-2 table  dinv^2 * (relu(G1) @ W2)  is computed per dest shard
    compactly, AllGathered (6.4MB), and expanded into the strided table.
"""

import os
import sys

if "/opt/trn_rl_repo" not in sys.path:
    sys.path.insert(0, "/opt/trn_rl_repo")

import numpy as np

from concourse import bass, bacc, mybir, tile
from concourse.bass_utils import run_bass_kernel_spmd

P = 128
NCORES = 8
WIN = 32768
CHUNK_T = int(os.environ.get("BG_CHUNK", "14"))  # dest tiles per gather chunk
TROW = 128    # table row stride in fp16 elements (= 256B)
POOL_SEL = int(os.environ.get("BG_POOLSEL", "5"))  # every POOL_SEL-th selector on Pool

F32 = mybir.dt.float32
F16 = mybir.dt.float16
I16 = mybir.dt.int16


def cdiv(a, b):
    return -(-a // b)


class Cfg:
    pass


def raw_gather(nc, out_ap, in_ap, idxs_ap, num_idxs, elem_size):
    """InstDMAGatherAnt with arbitrary elem_size (bytes%256 need not hold);
    row stride fixed at 256B (stride_bytes_256=1)."""
    gp = nc.gpsimd
    _in_ap = gp.lower_ap_dma(in_ap, for_custom_bir_dma=True)
    _idxs_ap = gp.lower_ap(idxs_ap)
    _out_ap = gp.lower_ap(out_ap)
    return gp.add_instruction(
        mybir.InstDMAGatherAnt(
            name=nc.get_next_instruction_name(),
            ins=[*_in_ap, _idxs_ap, gp.lower_val_access(gp.to_reg(num_idxs))],
            outs=[_out_ap],
            transpose=False,
            num_idxs=num_idxs,
            elem_size=elem_size,
            stride_bytes_256=1,
            gen_mode=0,
            single_packet=False,
            queue_num=0,
            sbuf_tokens_per_rank=0,
            sbuf_free_dim_per_rank=0,
            sbuf_free_dim_pad_per_rank=0,
            sbuf_byte_offset=0,
        )
    )


def prep(x, edge_index, W1, b1, W2, b2):
    """Host-side layout prep (shard, sort, window-group, pad)."""
    x = np.asarray(x, dtype=np.float32)
    W1 = np.asarray(W1, dtype=np.float32)
    b1 = np.asarray(b1, dtype=np.float32)
    W2 = np.asarray(W2, dtype=np.float32)
    b2 = np.asarray(b2, dtype=np.float32)

    N, FIN = x.shape
    FH = W1.shape[1]
    FO = W2.shape[1]

    cfg = Cfg()
    cfg.N, cfg.FIN, cfg.FH, cfg.FO = N, FIN, FH, FO
    SHARD_T = cdiv(N, P * NCORES)
    NT = SHARD_T * NCORES
    PAD_N = NT * P
    SHARD_N = SHARD_T * P
    cfg.SHARD_T, cfg.NT, cfg.PAD_N, cfg.SHARD_N = SHARD_T, NT, PAD_N, SHARD_N
    NW = cdiv(PAD_N, WIN)
    cfg.NW = NW
    cfg.win_rows = [min(PAD_N, (w + 1) * WIN) - w * WIN for w in range(NW)]
    cfg.use_b1 = bool(np.any(b1))
    cfg.use_b2 = bool(np.any(b2))

    row = np.asarray(edge_index[0], dtype=np.int64)
    col = np.asarray(edge_index[1], dtype=np.int64)
    loops = np.arange(N, dtype=np.int64)
    src_all = np.concatenate([row, loops])
    dst_all = np.concatenate([col, loops])

    # sort edges by (dest tile, window, dst)
    tile_of = dst_all // P
    win_of = src_all // WIN
    key = (tile_of * NW + win_of) * np.int64(PAD_N) + dst_all
    order = np.argsort(key, kind="stable")
    src_s = src_all[order]
    dst_s = dst_all[order]
    tw_s = (tile_of * NW + win_of)[order]

    # counts per (global tile, window) -> shared nblk (max over cores)
    cnts = np.bincount(tw_s, minlength=NT * NW).reshape(NCORES, SHARD_T, NW)
    nblk = cdiv(cnts, P).max(axis=0)  # [SHARD_T, NW]
    cfg.nblk = nblk

    # chunks of dest tiles
    chunks = []
    j = 0
    while j < SHARD_T:
        chunks.append((j, min(j + CHUNK_T, SHARD_T)))
        j += CHUNK_T
    cfg.chunks = chunks

    # slot layout (shared): for chunk: for w: for j in chunk
    slot_start = np.zeros((SHARD_T, NW), dtype=np.int64)
    gathers = []  # per chunk: (w, slot0, nslots)
    blocks_of_tile = [[] for _ in range(SHARD_T)]
    chunk_slot0 = []
    s = 0
    for (j0, j1) in chunks:
        chunk_slot0.append(s)
        glist = []
        for w in range(NW):
            sw0 = s
            for j in range(j0, j1):
                slot_start[j, w] = s
                nb = int(nblk[j, w])
                blocks_of_tile[j].extend(range(s // P, s // P + nb))
                s += nb * P
            if s > sw0:
                glist.append((w, sw0, s - sw0))
        gathers.append(glist)
    SLOTS = s
    NB = SLOTS // P
    cfg.SLOTS, cfg.NB = SLOTS, NB
    cfg.gathers = gathers
    cfg.chunk_slot0 = chunk_slot0
    cfg.blocks_of_tile = blocks_of_tile

    # degree (incl self-loop) -> dinv, computed on host
    deg_edges = np.bincount(col, minlength=PAD_N).astype(np.float64)
    deg = deg_edges + 1.0
    deg[N:] = 1.0
    dinv = (deg ** -0.5).astype(np.float32)
    dinv[N:] = 0.0

    # x^T fp16 pre-scaled by dinv[src], padded (plain node order; the
    # table write uses a (m p)-order 3D access pattern, no permute needed)
    xT_perm = np.zeros((FIN, PAD_N), dtype=np.float16)
    xT_perm[:, :N] = (x * dinv[:N, None]).T.astype(np.float16)

    iota = np.broadcast_to(np.arange(P, dtype=np.float16)[None, :], (P, P)).copy()
    W1h = W1.astype(np.float16)          # [FIN, FH] unpadded
    W2h = W2.astype(np.float16)          # [FH, FO]
    b2rep = np.broadcast_to(b2[None, :], (P, FO)).copy()
    b1r = b1[None, :].copy()

    rp_tiles = np.searchsorted(tw_s, np.arange(NT * NW + 1))

    in_maps = []
    for k in range(NCORES):
        idx_flat = np.zeros(SLOTS, dtype=np.int16)
        col_flat = np.full(SLOTS, -1.0, dtype=np.float32)
        for j in range(SHARD_T):
            t = k * SHARD_T + j
            for w in range(NW):
                a, b = rp_tiles[t * NW + w], rp_tiles[t * NW + w + 1]
                n = b - a
                if n == 0:
                    continue
                s0 = slot_start[j, w]
                idx_flat[s0:s0 + n] = (src_s[a:b] - w * WIN).astype(np.int16)
                col_flat[s0:s0 + n] = (dst_s[a:b] - t * P).astype(np.float32)
        idx16 = np.tile(idx_flat.reshape(-1, 16).T, (NCORES, 1))
        col_arr = col_flat.reshape(NB, P).T.copy()

        gn = (k * SHARD_N + np.arange(SHARD_N)).reshape(SHARD_T, P).T
        dg = dinv[gn]
        in_map = {
            "xT": xT_perm,
            "w1h": W1h,
            "w2h": W2h,
            "iota": iota,
            "idx16": idx16,
            "colv": col_arr,
            "dinv_g": dg.copy(),
            "dinv_gsq": (dg * dg).copy(),
        }
        if cfg.use_b2:
            in_map["b2rep"] = b2rep
        if cfg.use_b1:
            rn = k * SHARD_N + np.arange(SHARD_N)
            in_map["b1r"] = b1r
            # sqrt(deg) = 1/dinv (safe: pad nodes never relu'd into output)
            sq = np.where(dinv[rn] > 0, 1.0 / np.maximum(dinv[rn], 1e-30), 0.0)
            in_map["sqd_r"] = sq[None, :].astype(np.float32).copy()
        in_maps.append(in_map)

    return cfg, in_maps


def build_program(cfg):
    FIN, FH, FO = cfg.FIN, cfg.FH, cfg.FO
    NT, SHARD_T, PAD_N, SHARD_N = cfg.NT, cfg.SHARD_T, cfg.PAD_N, cfg.SHARD_N
    NW, NB, SLOTS = cfg.NW, cfg.NB, cfg.SLOTS

    BUF = int(os.environ.get("BG_BUF", "0"))
    nc = bacc.Bacc(
        "TRN2", target_bir_lowering=False, debug=False, num_devices=NCORES
    )

    xT_in = nc.dram_tensor("xT", [FIN, PAD_N], F16, kind="ExternalInput").ap()
    w1h_in = nc.dram_tensor("w1h", [FIN, FH], F16, kind="ExternalInput").ap()
    w2h_in = nc.dram_tensor("w2h", [FH, FO], F16, kind="ExternalInput").ap()
    iota_in = nc.dram_tensor("iota", [P, P], F16, kind="ExternalInput").ap()
    idx_in = nc.dram_tensor("idx16", [P, SLOTS // 16], I16, kind="ExternalInput").ap()
    col_in = nc.dram_tensor("colv", [P, NB], F32, kind="ExternalInput").ap()
    dinv_g_in = nc.dram_tensor("dinv_g", [P, SHARD_T], F32, kind="ExternalInput").ap()
    dinv_gsq_in = nc.dram_tensor(
        "dinv_gsq", [P, SHARD_T], F32, kind="ExternalInput"
    ).ap()
    if cfg.use_b2:
        b2rep_in = nc.dram_tensor("b2rep", [P, FO], F32, kind="ExternalInput").ap()
    if cfg.use_b1:
        b1r_in = nc.dram_tensor("b1r", [1, FH], F32, kind="ExternalInput").ap()
        sqd_in = nc.dram_tensor("sqd_r", [1, SHARD_N], F32, kind="ExternalInput").ap()

    table1 = nc.dram_tensor("table1", [PAD_N, TROW], F16, kind="Internal").ap()
    t2c = nc.dram_tensor("t2c", [SHARD_N, FO], F16, kind="Internal").ap()
    t2full = nc.dram_tensor(
        "t2full", [PAD_N, FO], F16, kind="Internal", addr_space="Shared"
    ).ap()
    table2 = nc.dram_tensor("table2", [PAD_N, TROW], F16, kind="Internal").ap()
    out = nc.dram_tensor("out", [SHARD_N, FO], F32, kind="ExternalOutput").ap()

    stage = os.environ.get("BASSGNN_STAGE", "full")
    sel_cnt = [0]

    with tile.TileContext(nc) as tc:
        with tc.tile_pool(name="const", bufs=1) as cpool:
            w1_t = cpool.tile([FIN, FH], F16)
            nc.sync.dma_start(out=w1_t[:], in_=w1h_in[:, :])
            w2_t = cpool.tile([FH, FO], F16)
            nc.sync.dma_start(out=w2_t[:], in_=w2h_in[:, :])
            iota_t = cpool.tile([P, P], F16)
            nc.sync.dma_start(out=iota_t[:], in_=iota_in[:, :])
            col_t = cpool.tile([P, NB], F32)
            nc.sync.dma_start(out=col_t[:], in_=col_in[:, :])
            idx_t = cpool.tile([P, SLOTS // 16], I16)
            nc.sync.dma_start(out=idx_t[:], in_=idx_in[:, :])
            dinv_g = cpool.tile([P, SHARD_T], F32)
            nc.sync.dma_start(out=dinv_g[:], in_=dinv_g_in[:, :])
            dinv_gsq = cpool.tile([P, SHARD_T], F32)
            nc.sync.dma_start(out=dinv_gsq[:], in_=dinv_gsq_in[:, :])
            if cfg.use_b2:
                b2_t = cpool.tile([P, FO], F32)
                nc.sync.dma_start(out=b2_t[:], in_=b2rep_in[:, :])
            if cfg.use_b1:
                b1_t = cpool.tile([1, FH], F32)
                nc.sync.dma_start(out=b1_t[:], in_=b1r_in[:, :])
                sqd_t = cpool.tile([1, SHARD_N], F32)
                nc.sync.dma_start(out=sqd_t[:], in_=sqd_in[:, :])

            # ---- Phase B: table1 = (dinv*x) @ W1, full table per core ----
            # 16 node-tiles per group; 8 matmul outputs packed per PSUM bank
            # ([P, 8*FH] fp32 = 2KB); one fp32->fp16 copy per bank,
            # alternating Activation / DVE.
            with (
                tc.tile_pool(name="phb", bufs=3 + BUF) as bpool,
                tc.tile_pool(name="phb_st", bufs=3 + BUF) as stpool,
                tc.tile_pool(name="phb_ps", bufs=4, space="PSUM") as bpsum,
            ):
                GT = 16
                assert NT % GT == 0
                BPB = 512 // FH   # matmul tiles per PSUM bank
                for g8 in range(NT // GT):
                    xt8 = bpool.tile([P, GT * P], F16, tag="xt")
                    nc.sync.dma_start(
                        out=xt8[:], in_=xT_in[:, g8 * GT * P:(g8 + 1) * GT * P]
                    )
                    st8 = stpool.tile([P, GT * FH], F16, tag="st")
                    for bk in range(GT // BPB):
                        hp = bpsum.tile([P, BPB * FH], F32, tag="hp")
                        for i in range(BPB):
                            t = bk * BPB + i
                            nc.tensor.matmul(
                                out=hp[:, i * FH:(i + 1) * FH],
                                lhsT=xt8[:, t * P:(t + 1) * P],
                                rhs=w1_t[:], start=True, stop=True,
                            )
                        dst_sl = st8[:, bk * BPB * FH:(bk + 1) * BPB * FH]
                        if bk % 2 == 0:
                            nc.scalar.activation(
                                out=dst_sl, in_=hp[:],
                                func=mybir.ActivationFunctionType.Copy,
                            )
                        else:
                            nc.vector.tensor_copy(out=dst_sl, in_=hp[:])
                    nc.sync.dma_start(
                        out=table1[
                            g8 * GT * P:(g8 + 1) * GT * P, :FH
                        ].rearrange("(m p) f -> p m f", p=P),
                        in_=st8[:].rearrange("p (m f) -> p m f", m=GT),
                    )

            (None if os.environ.get("BG_NOBAR") == "1"
                 else tc.strict_bb_all_engine_barrier())

            # ---- aggregation over edges ----
            def build_sel(spool, b):
                """One-hot selector for block b; split across DVE/Pool."""
                s_t = spool.tile([P, P], F16, tag="s")
                eng = (
                    nc.gpsimd
                    if POOL_SEL > 0 and sel_cnt[0] % POOL_SEL == POOL_SEL - 1
                    else nc.vector
                )
                sel_cnt[0] += 1
                eng.tensor_scalar(
                    out=s_t[:], in0=iota_t[:],
                    scalar1=col_t[:, b:b + 1], scalar2=None,
                    op0=mybir.AluOpType.is_equal,
                )
                return s_t

            def agg_layer(layer):
                tbl = table1 if layer == 1 else table2
                FA = FH if layer == 1 else FO
                nchunks = len(cfg.chunks)
                with (
                    tc.tile_pool(name=f"msg{layer}", bufs=2) as mpool,
                    tc.tile_pool(name=f"s{layer}", bufs=4 + 2 * BUF) as spool,
                    tc.tile_pool(name=f"work{layer}", bufs=3 + BUF) as wpool,
                    tc.tile_pool(name=f"ps{layer}", bufs=4 + (1 if BUF else 0), space="PSUM") as ppool,
                    tc.tile_pool(name=f"ps{layer}b", bufs=2 + (1 if BUF else 0), space="PSUM") as qpool,
                ):
                    def issue_gathers(ci):
                        sc0 = cfg.chunk_slot0[ci]
                        j0, j1 = cfg.chunks[ci]
                        cslots = sum(
                            int(cfg.nblk[j, w]) * P
                            for j in range(j0, j1) for w in range(NW)
                        )
                        ckb = cslots // P
                        msg = mpool.tile([P, ckb * FA], F16, tag="msg")
                        for (w, sw0, nw_slots) in cfg.gathers[ci]:
                            bw0 = (sw0 - sc0) // P
                            nbw = nw_slots // P
                            raw_gather(
                                nc,
                                out_ap=msg[
                                    :, bw0 * FA:(bw0 + nbw) * FA
                                ].rearrange("p (b f) -> p b f", f=FA),
                                in_ap=tbl[
                                    w * WIN: w * WIN + cfg.win_rows[w], :FA
                                ],
                                idxs_ap=idx_t[
                                    :, sw0 // 16: (sw0 + nw_slots) // 16
                                ],
                                num_idxs=nw_slots,
                                elem_size=FA,
                            )
                        return msg

                    msgs = {0: issue_gathers(0)}
                    for ci, (j0, j1) in enumerate(cfg.chunks):
                        if ci + 1 < nchunks:
                            msgs[ci + 1] = issue_gathers(ci + 1)
                        msg = msgs.pop(ci)
                        sc0 = cfg.chunk_slot0[ci]
                        if layer == 1:
                            st2big = wpool.tile([P, (j1 - j0) * FO], F16, tag="st2b")
                        else:
                            otbig = wpool.tile([P, (j1 - j0) * FO], F32, tag="otb")
                            o1big = wpool.tile([P, (j1 - j0) * FO], F32, tag="o1b")
                        for j in range(j0, j1):
                            blocks = cfg.blocks_of_tile[j]
                            nb = len(blocks)
                            ti = j - j0
                            if layer == 1:
                                gt = ppool.tile([FA, P], F32, tag="gt")
                                for i, b in enumerate(blocks):
                                    bl = b - sc0 // P
                                    s_t = build_sel(spool, b)
                                    nc.tensor.matmul(
                                        out=gt[:],
                                        lhsT=msg[:, bl * FA:(bl + 1) * FA],
                                        rhs=s_t[:],
                                        start=(i == 0),
                                        stop=(i == nb - 1 and not cfg.use_b1),
                                    )
                                if cfg.use_b1:
                                    nc.tensor.matmul(
                                        out=gt[:],
                                        lhsT=b1_t[:, :],
                                        rhs=sqd_t[:, j * P:(j + 1) * P],
                                        start=False,
                                        stop=True,
                                    )
                                r1 = wpool.tile([FH, P], F16, tag="r1")
                                nc.scalar.activation(
                                    out=r1[:], in_=gt[:],
                                    func=mybir.ActivationFunctionType.Relu,
                                )
                                h2p = qpool.tile([P, FO], F32, tag="h2")
                                nc.tensor.matmul(
                                    out=h2p[:], lhsT=r1[:], rhs=w2_t[:],
                                    start=True, stop=True,
                                )
                                nc.scalar.activation(
                                    out=st2big[:, ti * FO:(ti + 1) * FO],
                                    in_=h2p[:],
                                    func=mybir.ActivationFunctionType.Copy,
                                    scale=dinv_gsq[:, j:j + 1],
                                )
                            else:
                                # flipped: G[128 dest, FO] += S.T @ msg
                                gt2 = ppool.tile([P, FO], F32, tag="gt2")
                                for i, b in enumerate(blocks):
                                    bl = b - sc0 // P
                                    s_t = build_sel(spool, b)
                                    nc.tensor.matmul(
                                        out=gt2[:],
                                        lhsT=s_t[:],
                                        rhs=msg[:, bl * FA:(bl + 1) * FA],
                                        start=(i == 0),
                                        stop=(i == nb - 1),
                                    )
                                o1 = o1big[:, ti * FO:(ti + 1) * FO]
                                nc.scalar.activation(
                                    out=o1, in_=gt2[:],
                                    func=mybir.ActivationFunctionType.Copy,
                                    scale=dinv_g[:, j:j + 1],
                                )
                                if cfg.use_b2:
                                    nc.vector.tensor_tensor(
                                        out=o1, in0=o1, in1=b2_t[:],
                                        op=mybir.AluOpType.add,
                                    )
                        if layer == 2:
                            # batched log_softmax over the chunk's tiles
                            nt = j1 - j0
                            o3 = o1big[:].rearrange("p (t f) -> p t f", t=nt)
                            nm4 = wpool.tile([P, nt], F32, tag="nm4")
                            nc.vector.tensor_reduce(
                                out=nm4[:], in_=o3,
                                axis=mybir.AxisListType.X,
                                op=mybir.AluOpType.max, negate=True,
                            )
                            nm4b = nm4[:].rearrange(
                                "p (t one) -> p t one", one=1
                            ).to_broadcast([P, nt, FO])
                            o2b = wpool.tile([P, nt * FO], F32, tag="o2b")
                            nc.vector.tensor_tensor(
                                out=o2b[:].rearrange("p (t f) -> p t f", t=nt),
                                in0=o3, in1=nm4b, op=mybir.AluOpType.add,
                            )
                            e4 = wpool.tile([P, nt * FO], F32, tag="e4")
                            nc.scalar.activation(
                                out=e4[:], in_=o2b[:],
                                func=mybir.ActivationFunctionType.Exp,
                            )
                            ss4 = wpool.tile([P, nt], F32, tag="ss4")
                            nc.vector.tensor_reduce(
                                out=ss4[:],
                                in_=e4[:].rearrange("p (t f) -> p t f", t=nt),
                                axis=mybir.AxisListType.X,
                                op=mybir.AluOpType.add,
                            )
                            ls4 = wpool.tile([P, nt], F32, tag="ls4")
                            nc.scalar.activation(
                                out=ls4[:], in_=ss4[:],
                                func=mybir.ActivationFunctionType.Ln,
                            )
                            ls4b = ls4[:].rearrange(
                                "p (t one) -> p t one", one=1
                            ).to_broadcast([P, nt, FO])
                            nc.vector.tensor_tensor(
                                out=otbig[:].rearrange("p (t f) -> p t f", t=nt),
                                in0=o2b[:].rearrange("p (t f) -> p t f", t=nt),
                                in1=ls4b, op=mybir.AluOpType.subtract,
                            )
                        if layer == 1:
                            nc.sync.dma_start(
                                out=t2c[
                                    j0 * P:j1 * P, :
                                ].rearrange("(t p) f -> p t f", p=P),
                                in_=st2big[:].rearrange(
                                    "p (t f) -> p t f", t=j1 - j0
                                ),
                            )
                        else:
                            nc.sync.dma_start(
                                out=out[
                                    j0 * P:j1 * P, :
                                ].rearrange("(t p) f -> p t f", p=P),
                                in_=otbig[:].rearrange(
                                    "p (t f) -> p t f", t=j1 - j0
                                ),
                            )

            if stage != "b":
                agg_layer(1)

            if stage in ("full", "nocoll"):
                (None if os.environ.get("BG_NOBAR") == "1"
                 else tc.strict_bb_all_engine_barrier())
                if stage == "full":
                    nc.gpsimd.collective_compute(
                        "AllGather",
                        mybir.AluOpType.bypass,
                        replica_groups=[list(range(NCORES))],
                        ins=[t2c[:, :]],
                        outs=[t2full[:, :]],
                    )
                # expand compact [PAD_N, FO] into strided table2[:, :FO]
                src_t = t2full if stage == "full" else t2c
                nrow = PAD_N if stage == "full" else SHARD_N
                with tc.tile_pool(name="expand", bufs=3) as epool:
                    ET = 64  # tiles per expand group
                    for g in range(0, nrow // P, ET):
                        ge = min(g + ET, nrow // P)
                        ex = epool.tile([P, (ge - g) * FO], F16, tag="ex")
                        nc.sync.dma_start(
                            out=ex[:],
                            in_=src_t[g * P:ge * P, :].rearrange(
                                "(t p) f -> p t f", p=P
                            ),
                        )
                        nc.sync.dma_start(
                            out=table2[g * P:ge * P, :FO].rearrange(
                                "(t p) f -> p t f", p=P
                            ),
                            in_=ex[:].rearrange("p (t f) -> p t f", t=ge - g),
                        )
                (None if os.environ.get("BG_NOBAR") == "1"
                 else tc.strict_bb_all_engine_barrier())
                agg_layer(2)

    nc.compile()
    return nc


_CACHE = {}
TRACE = False
LAST = None


def kernel(x, edge_index, W1, b1, W2, b2):
    global LAST
    x = np.asarray(x)
    N = x.shape[0]
    cfg, in_maps = prep(x, edge_index, W1, b1, W2, b2)
    key = (
        N, cfg.FIN, cfg.FH, cfg.FO, cfg.SLOTS, cfg.use_b1, cfg.use_b2,
        tuple(cfg.nblk.reshape(-1).tolist()),
    )
    if key not in _CACHE:
        _CACHE[key] = build_program(cfg)
    nc = _CACHE[key]
    try:
        res = run_bass_kernel_spmd(
            nc, in_maps, core_ids=list(range(NCORES)), trace=TRACE
        )
    except Exception:
        # transient device wedge (NRT_EXEC_UNIT_UNRECOVERABLE) -- retry once
        import time as _time
        _time.sleep(10)
        res = run_bass_kernel_spmd(
            nc, in_maps, core_ids=list(range(NCORES)), trace=TRACE
        )
    LAST = res
    outs = [res.results[k]["out"] for k in range(NCORES)]
    full = np.concatenate(outs, axis=0)[:N]
    return full.astype(np.float32)


# revision 17
# speedup vs baseline: 1.1141x; 1.1141x over previous
"""Trainium2 Bass kernel for a 2-layer GCN (nn_ClusterGNN):
    h1 = relu(gcn_conv(x, W1, b1)); out = log_softmax(gcn_conv(h1, W2, b2))

Strategy (8 NeuronCores, dest-node sharded):
  - The GCN symmetric norm factorizes: msg(r->c) = dinv[r]*dinv[c]*h[r].
    dinv[src] is folded into x on the host, the dinv[dst] factor is
    deferred into downstream per-partition scales, so edge aggregation
    is a pure unweighted segment-sum.
  - Segment-sum runs on the tensor engine: per 128-edge block, a one-hot
    selector S[e, d] = (col_local[e] == d) is built with one
    tensor_scalar(is_equal) in fp16 (split across DVE and Pool engines),
    then layer 1 accumulates G^T[f, 128] += msg[128, F].T @ S[128, 128]
    and layer 2 accumulates G[128, F] += S[128, 128].T @ msg[128, F]
    (selector as stationary weights; Ldweights is free, so the matmul
    cost is the output free dim: 32 instead of 128).
  - Edge messages come from raw InstDMAGatherAnt (fp16 rows, 128B for
    layer 1 / 64B for layer 2, 256B row stride) out of a per-core
    replicated table; int16 indices are window-local (32768-row windows).
    Gathers are issued one chunk ahead of compute to keep the DMA
    engines busy.
  - Layer-2 table  dinv^2 * (relu(G1) @ W2)  is computed per dest shard
    compactly, AllGathered (6.4MB), and expanded into the strided table.
"""

import os
import sys

if "/opt/trn_rl_repo" not in sys.path:
    sys.path.insert(0, "/opt/trn_rl_repo")

import numpy as np

from concourse import bass, bacc, mybir, tile
from concourse.bass_utils import run_bass_kernel_spmd

P = 128
NCORES = 8
WIN = 32768
CHUNK_T = int(os.environ.get("BG_CHUNK", "12"))  # dest tiles per gather chunk
TROW = 128    # table row stride in fp16 elements (= 256B)
POOL_SEL = int(os.environ.get("BG_POOLSEL", "4"))  # every POOL_SEL-th selector on Pool (0 = all DVE)
MBUF = int(os.environ.get("BG_MBUF", "3"))       # msg tile buffers (chunks in flight)

F32 = mybir.dt.float32
F16 = mybir.dt.float16
I16 = mybir.dt.int16


def cdiv(a, b):
    return -(-a // b)


class Cfg:
    pass


def raw_gather(nc, out_ap, in_ap, idxs_ap, num_idxs, elem_size):
    """InstDMAGatherAnt with arbitrary elem_size (bytes%256 need not hold);
    row stride fixed at 256B (stride_bytes_256=1)."""
    gp = nc.gpsimd
    _in_ap = gp.lower_ap_dma(in_ap, for_custom_bir_dma=True)
    _idxs_ap = gp.lower_ap(idxs_ap)
    _out_ap = gp.lower_ap(out_ap)
    return gp.add_instruction(
        mybir.InstDMAGatherAnt(
            name=nc.get_next_instruction_name(),
            ins=[*_in_ap, _idxs_ap, gp.lower_val_access(gp.to_reg(num_idxs))],
            outs=[_out_ap],
            transpose=False,
            num_idxs=num_idxs,
            elem_size=elem_size,
            stride_bytes_256=1,
            gen_mode=0,
            single_packet=False,
            queue_num=0,
            sbuf_tokens_per_rank=0,
            sbuf_free_dim_per_rank=0,
            sbuf_free_dim_pad_per_rank=0,
            sbuf_byte_offset=0,
        )
    )


def prep(x, edge_index, W1, b1, W2, b2):
    """Host-side layout prep (shard, sort, window-group, pad)."""
    x = np.asarray(x, dtype=np.float32)
    W1 = np.asarray(W1, dtype=np.float32)
    b1 = np.asarray(b1, dtype=np.float32)
    W2 = np.asarray(W2, dtype=np.float32)
    b2 = np.asarray(b2, dtype=np.float32)

    N, FIN = x.shape
    FH = W1.shape[1]
    FO = W2.shape[1]

    cfg = Cfg()
    cfg.N, cfg.FIN, cfg.FH, cfg.FO = N, FIN, FH, FO
    SHARD_T = cdiv(N, P * NCORES)
    NT = SHARD_T * NCORES
    PAD_N = NT * P
    SHARD_N = SHARD_T * P
    cfg.SHARD_T, cfg.NT, cfg.PAD_N, cfg.SHARD_N = SHARD_T, NT, PAD_N, SHARD_N
    NW = cdiv(PAD_N, WIN)
    cfg.NW = NW
    cfg.win_rows = [min(PAD_N, (w + 1) * WIN) - w * WIN for w in range(NW)]
    cfg.use_b1 = bool(np.any(b1))
    cfg.use_b2 = bool(np.any(b2))

    row = np.asarray(edge_index[0], dtype=np.int64)
    col = np.asarray(edge_index[1], dtype=np.int64)
    loops = np.arange(N, dtype=np.int64)
    src_all = np.concatenate([row, loops])
    dst_all = np.concatenate([col, loops])

    # sort edges by (dest tile, window, dst)
    tile_of = dst_all // P
    win_of = src_all // WIN
    key = (tile_of * NW + win_of) * np.int64(PAD_N) + dst_all
    order = np.argsort(key, kind="stable")
    src_s = src_all[order]
    dst_s = dst_all[order]
    tw_s = (tile_of * NW + win_of)[order]

    # counts per (global tile, window) -> shared nblk (max over cores)
    cnts = np.bincount(tw_s, minlength=NT * NW).reshape(NCORES, SHARD_T, NW)
    nblk = cdiv(cnts, P).max(axis=0)  # [SHARD_T, NW]
    cfg.nblk = nblk

    # chunks of dest tiles
    chunks = []
    j = 0
    while j < SHARD_T:
        chunks.append((j, min(j + CHUNK_T, SHARD_T)))
        j += CHUNK_T
    cfg.chunks = chunks

    # slot layout (shared): for chunk: for w: for j in chunk
    slot_start = np.zeros((SHARD_T, NW), dtype=np.int64)
    gathers = []  # per chunk: (w, slot0, nslots)
    blocks_of_tile = [[] for _ in range(SHARD_T)]
    chunk_slot0 = []
    s = 0
    for (j0, j1) in chunks:
        chunk_slot0.append(s)
        glist = []
        for w in range(NW):
            sw0 = s
            for j in range(j0, j1):
                slot_start[j, w] = s
                nb = int(nblk[j, w])
                blocks_of_tile[j].extend(range(s // P, s // P + nb))
                s += nb * P
            if s > sw0:
                glist.append((w, sw0, s - sw0))
        gathers.append(glist)
    SLOTS = s
    NB = SLOTS // P
    cfg.SLOTS, cfg.NB = SLOTS, NB
    cfg.gathers = gathers
    cfg.chunk_slot0 = chunk_slot0
    cfg.blocks_of_tile = blocks_of_tile

    # degree (incl self-loop) -> dinv, computed on host
    deg_edges = np.bincount(col, minlength=PAD_N).astype(np.float64)
    deg = deg_edges + 1.0
    deg[N:] = 1.0
    dinv = (deg ** -0.5).astype(np.float32)
    dinv[N:] = 0.0

    # x^T fp16 pre-scaled by dinv[src], padded (plain node order; the
    # table write uses a (m p)-order 3D access pattern, no permute needed)
    xT_perm = np.zeros((FIN, PAD_N), dtype=np.float16)
    xT_perm[:, :N] = (x * dinv[:N, None]).T.astype(np.float16)

    iota = np.broadcast_to(np.arange(P, dtype=np.float16)[None, :], (P, P)).copy()
    W1h = W1.astype(np.float16)          # [FIN, FH] unpadded
    W2h = W2.astype(np.float16)          # [FH, FO]
    b2rep = np.broadcast_to(b2[None, :], (P, FO)).copy()
    b1r = b1[None, :].copy()

    rp_tiles = np.searchsorted(tw_s, np.arange(NT * NW + 1))

    in_maps = []
    for k in range(NCORES):
        idx_flat = np.zeros(SLOTS, dtype=np.int16)
        col_flat = np.full(SLOTS, -1.0, dtype=np.float32)
        for j in range(SHARD_T):
            t = k * SHARD_T + j
            for w in range(NW):
                a, b = rp_tiles[t * NW + w], rp_tiles[t * NW + w + 1]
                n = b - a
                if n == 0:
                    continue
                s0 = slot_start[j, w]
                idx_flat[s0:s0 + n] = (src_s[a:b] - w * WIN).astype(np.int16)
                col_flat[s0:s0 + n] = (dst_s[a:b] - t * P).astype(np.float32)
        idx16 = np.tile(idx_flat.reshape(-1, 16).T, (NCORES, 1))
        col_arr = col_flat.reshape(NB, P).T.copy()

        gn = (k * SHARD_N + np.arange(SHARD_N)).reshape(SHARD_T, P).T
        dg = dinv[gn]
        in_map = {
            "xT": xT_perm,
            "w1h": W1h,
            "w2h": W2h,
            "iota": iota,
            "idx16": idx16,
            "colv": col_arr,
            "dinv_g": dg.copy(),
            "dinv_gsq": (dg * dg).copy(),
        }
        if cfg.use_b2:
            in_map["b2rep"] = b2rep
        if cfg.use_b1:
            rn = k * SHARD_N + np.arange(SHARD_N)
            in_map["b1r"] = b1r
            # sqrt(deg) = 1/dinv (safe: pad nodes never relu'd into output)
            sq = np.where(dinv[rn] > 0, 1.0 / np.maximum(dinv[rn], 1e-30), 0.0)
            in_map["sqd_r"] = sq[None, :].astype(np.float32).copy()
        in_maps.append(in_map)

    return cfg, in_maps


def build_program(cfg):
    FIN, FH, FO = cfg.FIN, cfg.FH, cfg.FO
    NT, SHARD_T, PAD_N, SHARD_N = cfg.NT, cfg.SHARD_T, cfg.PAD_N, cfg.SHARD_N
    NW, NB, SLOTS = cfg.NW, cfg.NB, cfg.SLOTS

    BUF = int(os.environ.get("BG_BUF", "0"))
    nc = bacc.Bacc(
        "TRN2", target_bir_lowering=False, debug=False, num_devices=NCORES
    )

    xT_in = nc.dram_tensor("xT", [FIN, PAD_N], F16, kind="ExternalInput").ap()
    w1h_in = nc.dram_tensor("w1h", [FIN, FH], F16, kind="ExternalInput").ap()
    w2h_in = nc.dram_tensor("w2h", [FH, FO], F16, kind="ExternalInput").ap()
    iota_in = nc.dram_tensor("iota", [P, P], F16, kind="ExternalInput").ap()
    idx_in = nc.dram_tensor("idx16", [P, SLOTS // 16], I16, kind="ExternalInput").ap()
    col_in = nc.dram_tensor("colv", [P, NB], F32, kind="ExternalInput").ap()
    dinv_g_in = nc.dram_tensor("dinv_g", [P, SHARD_T], F32, kind="ExternalInput").ap()
    dinv_gsq_in = nc.dram_tensor(
        "dinv_gsq", [P, SHARD_T], F32, kind="ExternalInput"
    ).ap()
    if cfg.use_b2:
        b2rep_in = nc.dram_tensor("b2rep", [P, FO], F32, kind="ExternalInput").ap()
    if cfg.use_b1:
        b1r_in = nc.dram_tensor("b1r", [1, FH], F32, kind="ExternalInput").ap()
        sqd_in = nc.dram_tensor("sqd_r", [1, SHARD_N], F32, kind="ExternalInput").ap()

    table1 = nc.dram_tensor("table1", [PAD_N, TROW], F16, kind="Internal").ap()
    t2c = nc.dram_tensor("t2c", [SHARD_N, FO], F16, kind="Internal").ap()
    t2full = nc.dram_tensor(
        "t2full", [PAD_N, FO], F16, kind="Internal", addr_space="Shared"
    ).ap()
    table2 = nc.dram_tensor("table2", [PAD_N, TROW], F16, kind="Internal").ap()
    out = nc.dram_tensor("out", [SHARD_N, FO], F32, kind="ExternalOutput").ap()

    stage = os.environ.get("BASSGNN_STAGE", "full")
    sel_cnt = [0]

    with tile.TileContext(nc) as tc:
        with tc.tile_pool(name="const", bufs=1) as cpool:
            w1_t = cpool.tile([FIN, FH], F16)
            nc.sync.dma_start(out=w1_t[:], in_=w1h_in[:, :])
            w2_t = cpool.tile([FH, FO], F16)
            nc.sync.dma_start(out=w2_t[:], in_=w2h_in[:, :])
            iota_t = cpool.tile([P, P], F16)
            nc.sync.dma_start(out=iota_t[:], in_=iota_in[:, :])
            col_t = cpool.tile([P, NB], F32)
            nc.sync.dma_start(out=col_t[:], in_=col_in[:, :])
            idx_t = cpool.tile([P, SLOTS // 16], I16)
            nc.sync.dma_start(out=idx_t[:], in_=idx_in[:, :])
            dinv_g = cpool.tile([P, SHARD_T], F32)
            nc.sync.dma_start(out=dinv_g[:], in_=dinv_g_in[:, :])
            dinv_gsq = cpool.tile([P, SHARD_T], F32)
            nc.sync.dma_start(out=dinv_gsq[:], in_=dinv_gsq_in[:, :])
            if cfg.use_b2:
                b2_t = cpool.tile([P, FO], F32)
                nc.sync.dma_start(out=b2_t[:], in_=b2rep_in[:, :])
            if cfg.use_b1:
                b1_t = cpool.tile([1, FH], F32)
                nc.sync.dma_start(out=b1_t[:], in_=b1r_in[:, :])
                sqd_t = cpool.tile([1, SHARD_N], F32)
                nc.sync.dma_start(out=sqd_t[:], in_=sqd_in[:, :])

            # ---- Phase B: table1 = (dinv*x) @ W1, full table per core ----
            # 16 node-tiles per group; 8 matmul outputs packed per PSUM bank
            # ([P, 8*FH] fp32 = 2KB); one fp32->fp16 copy per bank,
            # alternating Activation / DVE.
            with (
                tc.tile_pool(name="phb", bufs=3 + BUF) as bpool,
                tc.tile_pool(name="phb_st", bufs=3 + BUF) as stpool,
                tc.tile_pool(name="phb_ps", bufs=4, space="PSUM") as bpsum,
            ):
                GT = 16
                assert NT % GT == 0
                BPB = 512 // FH   # matmul tiles per PSUM bank
                for g8 in range(NT // GT):
                    xt8 = bpool.tile([P, GT * P], F16, tag="xt")
                    nc.sync.dma_start(
                        out=xt8[:], in_=xT_in[:, g8 * GT * P:(g8 + 1) * GT * P]
                    )
                    st8 = stpool.tile([P, GT * FH], F16, tag="st")
                    for bk in range(GT // BPB):
                        hp = bpsum.tile([P, BPB * FH], F32, tag="hp")
                        for i in range(BPB):
                            t = bk * BPB + i
                            nc.tensor.matmul(
                                out=hp[:, i * FH:(i + 1) * FH],
                                lhsT=xt8[:, t * P:(t + 1) * P],
                                rhs=w1_t[:], start=True, stop=True,
                            )
                        dst_sl = st8[:, bk * BPB * FH:(bk + 1) * BPB * FH]
                        if bk % 2 == 0:
                            nc.scalar.activation(
                                out=dst_sl, in_=hp[:],
                                func=mybir.ActivationFunctionType.Copy,
                            )
                        else:
                            nc.vector.tensor_copy(out=dst_sl, in_=hp[:])
                    nc.sync.dma_start(
                        out=table1[
                            g8 * GT * P:(g8 + 1) * GT * P, :FH
                        ].rearrange("(m p) f -> p m f", p=P),
                        in_=st8[:].rearrange("p (m f) -> p m f", m=GT),
                    )

            (None if os.environ.get("BG_NOBAR") == "1"
                 else tc.strict_bb_all_engine_barrier())

            # ---- aggregation over edges ----
            def build_sel(spool, b):
                """One-hot selector for block b; split across DVE/Pool."""
                s_t = spool.tile([P, P], F16, tag="s")
                eng = (
                    nc.gpsimd
                    if POOL_SEL > 0 and sel_cnt[0] % POOL_SEL == POOL_SEL - 1
                    else nc.vector
                )
                sel_cnt[0] += 1
                eng.tensor_scalar(
                    out=s_t[:], in0=iota_t[:],
                    scalar1=col_t[:, b:b + 1], scalar2=None,
                    op0=mybir.AluOpType.is_equal,
                )
                return s_t

            def agg_layer(layer):
                tbl = table1 if layer == 1 else table2
                FA = FH if layer == 1 else FO
                nchunks = len(cfg.chunks)
                SB = int(os.environ.get("BG_SBUF", "16"))
                with (
                    tc.tile_pool(name=f"msg{layer}", bufs=MBUF) as mpool,
                    tc.tile_pool(name=f"s{layer}", bufs=SB) as spool,
                    tc.tile_pool(name=f"work{layer}", bufs=3 + BUF) as wpool,
                    tc.tile_pool(name=f"ps{layer}", bufs=5, space="PSUM") as ppool,
                    tc.tile_pool(name=f"ps{layer}b", bufs=2, space="PSUM") as qpool,
                ):
                    def issue_gathers(ci):
                        sc0 = cfg.chunk_slot0[ci]
                        j0, j1 = cfg.chunks[ci]
                        cslots = sum(
                            int(cfg.nblk[j, w]) * P
                            for j in range(j0, j1) for w in range(NW)
                        )
                        ckb = cslots // P
                        msg = mpool.tile([P, ckb * FA], F16, tag="msg")
                        for (w, sw0, nw_slots) in cfg.gathers[ci]:
                            bw0 = (sw0 - sc0) // P
                            nbw = nw_slots // P
                            raw_gather(
                                nc,
                                out_ap=msg[
                                    :, bw0 * FA:(bw0 + nbw) * FA
                                ].rearrange("p (b f) -> p b f", f=FA),
                                in_ap=tbl[
                                    w * WIN: w * WIN + cfg.win_rows[w], :FA
                                ],
                                idxs_ap=idx_t[
                                    :, sw0 // 16: (sw0 + nw_slots) // 16
                                ],
                                num_idxs=nw_slots,
                                elem_size=FA,
                            )
                        return msg

                    msgs = {}
                    for ci0 in range(min(MBUF - 1, nchunks)):
                        msgs[ci0] = issue_gathers(ci0)
                    for ci, (j0, j1) in enumerate(cfg.chunks):
                        nxt = ci + MBUF - 1
                        if nxt < nchunks:
                            msgs[nxt] = issue_gathers(nxt)
                        msg = msgs.pop(ci)
                        sc0 = cfg.chunk_slot0[ci]
                        if layer == 1:
                            st2big = wpool.tile([P, (j1 - j0) * FO], F16, tag="st2b")
                        else:
                            otbig = wpool.tile([P, (j1 - j0) * FO], F32, tag="otb")
                            o1big = wpool.tile([P, (j1 - j0) * FO], F32, tag="o1b")
                        for j in range(j0, j1):
                            blocks = cfg.blocks_of_tile[j]
                            nb = len(blocks)
                            ti = j - j0
                            if layer == 1:
                                gt = ppool.tile([FA, P], F32, tag="gt")
                                for i, b in enumerate(blocks):
                                    bl = b - sc0 // P
                                    s_t = build_sel(spool, b)
                                    nc.tensor.matmul(
                                        out=gt[:],
                                        lhsT=msg[:, bl * FA:(bl + 1) * FA],
                                        rhs=s_t[:],
                                        start=(i == 0),
                                        stop=(i == nb - 1 and not cfg.use_b1),
                                    )
                                if cfg.use_b1:
                                    nc.tensor.matmul(
                                        out=gt[:],
                                        lhsT=b1_t[:, :],
                                        rhs=sqd_t[:, j * P:(j + 1) * P],
                                        start=False,
                                        stop=True,
                                    )
                                r1 = wpool.tile([FH, P], F16, tag="r1")
                                nc.scalar.activation(
                                    out=r1[:], in_=gt[:],
                                    func=mybir.ActivationFunctionType.Relu,
                                )
                                h2p = qpool.tile([P, FO], F32, tag="h2")
                                nc.tensor.matmul(
                                    out=h2p[:], lhsT=r1[:], rhs=w2_t[:],
                                    start=True, stop=True,
                                )
                                nc.scalar.activation(
                                    out=st2big[:, ti * FO:(ti + 1) * FO],
                                    in_=h2p[:],
                                    func=mybir.ActivationFunctionType.Copy,
                                    scale=dinv_gsq[:, j:j + 1],
                                )
                            else:
                                # flipped: G[128 dest, FO] += S.T @ msg
                                gt2 = ppool.tile([P, FO], F32, tag="gt2")
                                for i, b in enumerate(blocks):
                                    bl = b - sc0 // P
                                    s_t = build_sel(spool, b)
                                    nc.tensor.matmul(
                                        out=gt2[:],
                                        lhsT=s_t[:],
                                        rhs=msg[:, bl * FA:(bl + 1) * FA],
                                        start=(i == 0),
                                        stop=(i == nb - 1),
                                    )
                                o1 = o1big[:, ti * FO:(ti + 1) * FO]
                                nc.scalar.activation(
                                    out=o1, in_=gt2[:],
                                    func=mybir.ActivationFunctionType.Copy,
                                    scale=dinv_g[:, j:j + 1],
                                )
                                if cfg.use_b2:
                                    nc.vector.tensor_tensor(
                                        out=o1, in0=o1, in1=b2_t[:],
                                        op=mybir.AluOpType.add,
                                    )
                        if layer == 2:
                            # batched log_softmax over the chunk's tiles
                            nt = j1 - j0
                            o3 = o1big[:].rearrange("p (t f) -> p t f", t=nt)
                            nm4 = wpool.tile([P, nt], F32, tag="nm4")
                            nc.vector.tensor_reduce(
                                out=nm4[:], in_=o3,
                                axis=mybir.AxisListType.X,
                                op=mybir.AluOpType.max, negate=True,
                            )
                            nm4b = nm4[:].rearrange(
                                "p (t one) -> p t one", one=1
                            ).to_broadcast([P, nt, FO])
                            o2b = wpool.tile([P, nt * FO], F32, tag="o2b")
                            nc.vector.tensor_tensor(
                                out=o2b[:].rearrange("p (t f) -> p t f", t=nt),
                                in0=o3, in1=nm4b, op=mybir.AluOpType.add,
                            )
                            e4 = wpool.tile([P, nt * FO], F32, tag="e4")
                            nc.scalar.activation(
                                out=e4[:], in_=o2b[:],
                                func=mybir.ActivationFunctionType.Exp,
                            )
                            ss4 = wpool.tile([P, nt], F32, tag="ss4")
                            nc.vector.tensor_reduce(
                                out=ss4[:],
                                in_=e4[:].rearrange("p (t f) -> p t f", t=nt),
                                axis=mybir.AxisListType.X,
                                op=mybir.AluOpType.add,
                            )
                            ls4 = wpool.tile([P, nt], F32, tag="ls4")
                            nc.scalar.activation(
                                out=ls4[:], in_=ss4[:],
                                func=mybir.ActivationFunctionType.Ln,
                            )
                            ls4b = ls4[:].rearrange(
                                "p (t one) -> p t one", one=1
                            ).to_broadcast([P, nt, FO])
                            nc.vector.tensor_tensor(
                                out=otbig[:].rearrange("p (t f) -> p t f", t=nt),
                                in0=o2b[:].rearrange("p (t f) -> p t f", t=nt),
                                in1=ls4b, op=mybir.AluOpType.subtract,
                            )
                        if layer == 1:
                            nc.sync.dma_start(
                                out=t2c[
                                    j0 * P:j1 * P, :
                                ].rearrange("(t p) f -> p t f", p=P),
                                in_=st2big[:].rearrange(
                                    "p (t f) -> p t f", t=j1 - j0
                                ),
                            )
                        else:
                            nc.sync.dma_start(
                                out=out[
                                    j0 * P:j1 * P, :
                                ].rearrange("(t p) f -> p t f", p=P),
                                in_=otbig[:].rearrange(
                                    "p (t f) -> p t f", t=j1 - j0
                                ),
                            )

            if stage != "b":
                agg_layer(1)

            if stage in ("full", "nocoll"):
                (None if os.environ.get("BG_NOBAR") == "1"
                 else tc.strict_bb_all_engine_barrier())
                if stage == "full":
                    nc.gpsimd.collective_compute(
                        "AllGather",
                        mybir.AluOpType.bypass,
                        replica_groups=[list(range(NCORES))],
                        ins=[t2c[:, :]],
                        outs=[t2full[:, :]],
                    )
                # expand compact [PAD_N, FO] into strided table2[:, :FO]
                src_t = t2full if stage == "full" else t2c
                nrow = PAD_N if stage == "full" else SHARD_N
                with tc.tile_pool(name="expand", bufs=3) as epool:
                    ET = 64  # tiles per expand group
                    for g in range(0, nrow // P, ET):
                        ge = min(g + ET, nrow // P)
                        ex = epool.tile([P, (ge - g) * FO], F16, tag="ex")
                        nc.sync.dma_start(
                            out=ex[:],
                            in_=src_t[g * P:ge * P, :].rearrange(
                                "(t p) f -> p t f", p=P
                            ),
                        )
                        nc.sync.dma_start(
                            out=table2[g * P:ge * P, :FO].rearrange(
                                "(t p) f -> p t f", p=P
                            ),
                            in_=ex[:].rearrange("p (t f) -> p t f", t=ge - g),
                        )
                (None if os.environ.get("BG_NOBAR") == "1"
                 else tc.strict_bb_all_engine_barrier())
                agg_layer(2)

    nc.compile()
    return nc


_CACHE = {}
TRACE = False
LAST = None


def kernel(x, edge_index, W1, b1, W2, b2):
    global LAST
    x = np.asarray(x)
    N = x.shape[0]
    cfg, in_maps = prep(x, edge_index, W1, b1, W2, b2)
    key = (
        N, cfg.FIN, cfg.FH, cfg.FO, cfg.SLOTS, cfg.use_b1, cfg.use_b2,
        tuple(cfg.nblk.reshape(-1).tolist()),
    )
    if key not in _CACHE:
        _CACHE[key] = build_program(cfg)
    nc = _CACHE[key]
    try:
        res = run_bass_kernel_spmd(
            nc, in_maps, core_ids=list(range(NCORES)), trace=TRACE
        )
    except Exception:
        # transient device wedge (NRT_EXEC_UNIT_UNRECOVERABLE) -- retry once
        import time as _time
        _time.sleep(10)
        res = run_bass_kernel_spmd(
            nc, in_maps, core_ids=list(range(NCORES)), trace=TRACE
        )
    LAST = res
    outs = [res.results[k]["out"] for k in range(NCORES)]
    full = np.concatenate(outs, axis=0)[:N]
    return full.astype(np.float32)
